# revision 16
# baseline (speedup 1.0000x reference)
"""AFT (Attention-Free Transformer) distributed Bass kernel for 8 TRN2 NeuronCores.

Sharding: core = (batch n in 0..3) x (head-half g in 0..1). Each core projects
k/v for only its 4 heads (halving the projection matmul vs recompute), runs
the causal einsum for those heads over ALL 16 t-blocks, then a pairwise
AllGather (TOPSP/SDMA silicon, overlapped with einsum) exchanges aft halves
so each core can out-project its 8 owned t-blocks over all 8 heads.

SPMD symmetry: the einsum processes t-blocks in 16 slots = [send-set 8,
own-set 8]; per-parity block orders (OB_A/OB_B) are host-packed into the
w_aft slabs so both parities share one instruction stream with identical
padded causal extents (EQ). The out-proj contracts 12 tiles (4 local af +
8 AllGather tiles); host-zeroed Wo rows select the partner half per core.

exp(w_aft) (+ causal mask as exact zeros) is precomputed on host into the
slabs - no on-device exp for the einsum at all.

Per-core pipeline (matmuls bf16, fp32 PSUM):
  1. k/v proj (4 heads): k|v = xT.T @ [Wk|Wv][:, own cols] -> ek=exp(k),
     ekv=ek*v resident in SBUF.
  2. einsum per quad of 4 t-block slots: num/den[d,t] += (ekv|ek)[s,d].T @
     ew_slab[s, t-cols]; aft = num * (1/den) -> bf16.
  3. after send-set quads 0/1: DMA aft -> DRAM, chunked pairwise AllGather,
     DMA partner half -> SBUF (hidden under own-set einsum).
  4. out-proj: out[t,:] = sum_idx aft_tile_idx.T @ Wo_idx (12 tiles).

Self-contained: hardcodes all shapes for x[4,2048,1024], w_aft[8,2048,2048].
"""

import os

import numpy as np
import ml_dtypes

import concourse.bass as bass
import concourse.bacc as bacc
import concourse.mybir as mybir
import concourse.tile as tile

BF16 = ml_dtypes.bfloat16
P = 128
N_B, SEQ, DIM, H = 4, 2048, 1024, 8
HL = 4            # local heads per core
NT = 16           # 128-row t-blocks
HCOL = HL * P     # 512 k (or v) columns per core

# Per-parity t-block orders (from the baseline's balanced causal pairing):
# slots j and the EQ padded extents line up across parities so both cores
# share one instruction stream.
OB_A = [7, 4, 3, 0, 15, 12, 11, 8]
OB_B = [6, 5, 2, 1, 14, 13, 10, 9]
EQ8 = [8, 6, 4, 2, 16, 14, 12, 10]   # padded extent (s-tiles) per slot in a set
# 16 slots = send-set(8) + own-set(8); 4 quads of 4 slots.
EQ16 = EQ8 + EQ8
QUAD_ST = [8, 16, 8, 16]             # s-tile loop bound per quad


def _w_of(q, st):
    return P * sum(1 for j in range(4) if EQ16[4 * q + j] > st)


# Slab consumption order: (quad, s-tile) -> (W, col offset in wt)
SLABS = []
_off = 0
for _q in range(4):
    for _st in range(QUAD_ST[_q]):
        _W = _w_of(_q, _st)
        SLABS.append((_q, _st, _W, _off))
        _off += HL * _W
TOTC = _off  # 4 heads * 144 s-tile-blocks * 128 = 73728

LAST_EXEC_NS = None
LAST_RESULTS = None


def build_nc(has_bias):
    NIT = 9 if has_bias else 8   # k-tiles in the x^T contraction (+1 bias row)
    NHO = 13 if has_bias else 12  # out-proj contraction tiles (+1 bias row)
    SXT = NIT * 128
    F32 = mybir.dt.float32
    BF = mybir.dt.bfloat16
    F8 = mybir.dt.float8e4
    EXP = mybir.ActivationFunctionType.Exp
    RG = [[0, 1], [2, 3], [4, 5], [6, 7]]

    nc = bacc.Bacc("TRN2", target_bir_lowering=False, num_devices=8)
    xt_d = nc.declare_dram_parameter("xt", [16, P, SXT], BF, isOutput=False)
    wkv_d = nc.declare_dram_parameter("wkv", [NIT, P, 1024], BF, isOutput=False)
    wo_d = nc.declare_dram_parameter("wo", [P, NHO, 1024], BF, isOutput=False)
    wt_d = nc.declare_dram_parameter("wt", [P, TOTC], F8, isOutput=False)
    out_d = nc.declare_dram_parameter("out", [1024, 1024], F32, isOutput=True)

    with tile.TileContext(nc) as tc:
        with tc.tile_pool(name="res", bufs=1) as res, \
             tc.tile_pool(name="aftp", bufs=16) as aftp, \
             tc.tile_pool(name="wop", bufs=1) as wop, \
             tc.tile_pool(name="wkvp", bufs=1) as wkvp, \
             tc.tile_pool(name="wtr", bufs=10) as wtr, \
             tc.tile_pool(name="recp", bufs=2) as recp, \
             tc.tile_pool(name="outp", bufs=3) as outp, \
             tc.tile_pool(name="agd", bufs=4, space="DRAM") as agd:
            ek_sb = res.tile([P, 16, HCOL], BF, name="ek_sb")
            ekv_sb = res.tile([P, 16, HCOL], BF, name="ekv_sb")
            xt_sb = res.tile([P, 16, SXT], BF, name="xt_sb")
            ag_sb = res.tile([P, 8, 1024], BF, name="ag_sb")
            wo_sb = wop.tile([P, NHO, 1024], BF, name="wo_sb")
            wkv_sb = wkvp.tile([P, NIT, 1024], BF, name="wkv_sb")
            agin_a = agd.tile([512, 512], BF, name="agin_a")
            agin_b = agd.tile([512, 512], BF, name="agin_b")
            agout_a = agd.tile([1024, 512], BF, name="agout_a")
            agout_b = agd.tile([1024, 512], BF, name="agout_b")

            # first-needed inputs lead their queues: wkv[0] on sync,
            # xt[0] on gpsimd; wo (needed only in phase 3) is issued after
            # the phase-1 loop so it doesn't contend with startup DMAs.
            nc.gpsimd.dma_start(out=xt_sb[:, 0, :], in_=xt_d[0, :, :])
            for i in range(NIT):
                nc.sync.dma_start(out=wkv_sb[:, i, :], in_=wkv_d[i, :, :])
            ones_t = None
            if has_bias:
                ones_t = res.tile([P, P], BF, name="ones_t")
                nc.vector.memset(ones_t[:, :], 0.0)
                nc.vector.memset(ones_t[0:1, :], 1.0)

            # ---------------- phase 1: k/v projection (own 4 heads) --------
            with tc.tile_pool(name="pkv", bufs=6, space="PSUM") as pkv:
                for st in range(16):
                    if st > 0:
                        nc.sync.dma_start(out=xt_sb[:, st, :],
                                          in_=xt_d[st, :, :])
                    kp = pkv.tile([P, HCOL], F32, name="kp", tag="pkv")
                    vp = pkv.tile([P, HCOL], F32, name="vp", tag="pkv")
                    for it in range(NIT):
                        lh = xt_sb[:, st, it * 128:(it + 1) * 128]
                        s0 = it == 0
                        s1 = it == NIT - 1
                        nc.tensor.matmul(kp[:, :], lh,
                                         wkv_sb[:, it, 0:HCOL],
                                         start=s0, stop=s1)
                        nc.tensor.matmul(vp[:, :], lh,
                                         wkv_sb[:, it, HCOL:1024],
                                         start=s0, stop=s1)
                    nc.scalar.activation(ek_sb[:, st, :], kp[:, :], EXP)
                    nc.vector.tensor_mul(ekv_sb[:, st, :], vp[:, :],
                                         ek_sb[:, st, :])
                for i in range(NHO):
                    nc.scalar.dma_start(out=wo_sb[:, i, :], in_=wo_d[:, i, :])

            # ------------- phase 2: einsum + aft exchange ------------------
            aft = {}
            si = 0
            with tc.tile_pool(name="pe", bufs=8, space="PSUM") as pep:
                for q in range(4):
                    nd = []
                    for hh in range(HL):
                        nt = pep.tile([P, 512], F32, name="ps_n", tag="ps")
                        dn = pep.tile([P, 512], F32, name="ps_d", tag="ps")
                        nd.append((nt, dn))
                    for st in range(QUAD_ST[q]):
                        q_, st_, W, off = SLABS[si]
                        si += 1
                        assert (q_, st_) == (q, st)
                        slab = wtr.tile([P, 2048], F8, name="slab", tag="slab")
                        nc.sync.dma_start(out=slab[:, 0:HL * W],
                                          in_=wt_d[:, off:off + HL * W])
                        s0 = st == 0
                        s1 = st == QUAD_ST[q] - 1
                        for hh in range(HL):
                            nt, dn = nd[hh]
                            rhs = slab[:, hh * W:(hh + 1) * W]
                            nc.tensor.matmul(
                                nt[:, 0:W],
                                ekv_sb[:, st, hh * 128:(hh + 1) * 128],
                                rhs, start=s0, stop=s1)
                            nc.tensor.matmul(
                                dn[:, 0:W],
                                ek_sb[:, st, hh * 128:(hh + 1) * 128],
                                rhs, start=s0, stop=s1)
                    for hh in range(HL):
                        nt, dn = nd[hh]
                        rc = recp.tile([P, 512], F32, name="rc", tag="rc")
                        nc.vector.reciprocal_approx_fast(rc[:, :], dn[:, :])
                        af = aftp.tile([P, 512], BF, name="af", tag="af")
                        nc.vector.tensor_mul(af[:, :], nt[:, :], rc[:, :])
                        aft[(q, hh)] = af

            # Collectives are emitted AFTER the whole einsum: the tile
            # framework serializes later-emitted DMAs behind in-flight
            # collectives, so any slab DMA after an AG would stall the PE.
            # Data deps (af tiles -> agin -> AG -> agout -> ag_sb) still
            # order these correctly at runtime.
            for q in range(2):
                agin = agin_a if q == 0 else agin_b
                for hh in range(HL):
                    nc.scalar.dma_start(
                        out=agin[hh * 128:(hh + 1) * 128, :],
                        in_=aft[(q, hh)][:, :])
            for q in range(2):
                agin = agin_a if q == 0 else agin_b
                agout = agout_a if q == 0 else agout_b
                nc.gpsimd.collective_compute(
                    "AllGather", mybir.AluOpType.bypass,
                    replica_groups=RG,
                    ins=[agin.opt()], outs=[agout.opt()])
            for q in range(2):
                agout = agout_a if q == 0 else agout_b
                c0 = q * 512
                for half in range(2):
                    for hh in range(HL):
                        r0 = half * 512 + hh * 128
                        nc.scalar.dma_start(
                            out=ag_sb[:, half * 4 + hh, c0:c0 + 512],
                            in_=agout[r0:r0 + 128, :])

            # ---------------- phase 3: out-projection ----------------------
            with tc.tile_pool(name="po", bufs=2, space="PSUM") as pop:
                for tb in range(8):
                    q = 2 + tb // 4
                    jj = tb % 4
                    ops = pop.tile([P, 1024], F32, name="ps_o", tag="po")
                    for idx in range(NHO):
                        if idx < 4:
                            lh = aft[(q, idx)][:, jj * 128:(jj + 1) * 128]
                        elif idx < 12:
                            lh = ag_sb[:, idx - 4, tb * 128:(tb + 1) * 128]
                        else:
                            lh = ones_t[:, :]
                        s0 = idx == 0
                        s1 = idx == NHO - 1
                        nc.tensor.matmul(
                            ops[:, 0:512], lh, wo_sb[:, idx, 0:512],
                            start=s0, stop=s1)
                        nc.tensor.matmul(
                            ops[:, 512:1024], lh, wo_sb[:, idx, 512:1024],
                            start=s0, stop=s1)
                    osb = outp.tile([P, 1024], F32, name="osb", tag="osb")
                    nc.vector.tensor_copy(osb[:, :], ops[:, :])
                    nc.sync.dma_start(
                        out=out_d[tb * 128:(tb + 1) * 128, :], in_=osb[:, :])
    nc.compile()
    return nc


def pack_core(xn, Wk, bk, Wv, bv, ew, Wo, bo, g, has_bias):
    """Per-core input map (layout transforms + bf16 casts). ew = masked
    exp(w_aft) [8, T, S] fp32, computed once by the caller."""
    SEND = OB_B if g == 0 else OB_A
    OWN = OB_A if g == 0 else OB_B
    slots = SEND + OWN
    h0 = HL * g

    # x^T tiles: xt[st, p, it*128+ss] = x[st*128+ss, it*128+p]
    xr = xn.reshape(16, 128, 8, 128).transpose(0, 3, 2, 1)  # [st, p, it, ss]
    xt = np.ascontiguousarray(xr).reshape(16, 128, 1024)
    if has_bias:
        aug = np.zeros((16, 128, 128), np.float32)
        aug[:, 0, :] = 1.0
        xt = np.concatenate([xt, aug], axis=2)
    xt = xt.astype(BF16)

    wkv = np.concatenate([Wk[:, h0 * 128:(h0 + HL) * 128],
                          Wv[:, h0 * 128:(h0 + HL) * 128]],
                         axis=1).reshape(8, 128, 1024)
    if has_bias:
        aug = np.zeros((1, 128, 1024), np.float32)
        aug[0, 0, :] = np.concatenate([bk[h0 * 128:(h0 + HL) * 128],
                                       bv[h0 * 128:(h0 + HL) * 128]])
        wkv = np.concatenate([wkv, aug], axis=0)
    wkv = wkv.astype(BF16)

    # out-proj tiles: 0-3 own heads, 4-7 AG lo half (global h0-3),
    # 8-11 AG hi half (global h4-7); partner rows real, self-echo zero.
    wor = Wo.reshape(8, 128, 1024)
    wo = np.zeros((12, 128, 1024), np.float32)
    wo[0:4] = wor[h0:h0 + HL]
    if g == 0:
        wo[8:12] = wor[4:8]
    else:
        wo[4:8] = wor[0:4]
    if has_bias:
        aug = np.zeros((1, 128, 1024), np.float32)
        aug[0, 0, :] = bo
        wo = np.concatenate([wo, aug], axis=0)
    wo = np.ascontiguousarray(wo.transpose(1, 0, 2)).astype(BF16)

    # host-precomputed exp(w_aft) slabs, causally packed & transposed
    wt = np.empty((128, TOTC), np.float32)
    for (q, st, W, off) in SLABS:
        cnt = W // 128
        sub = np.zeros((128, HL, W), np.float32)
        sg = st * 128
        for j in range(cnt):
            b = slots[4 * q + j]
            t0 = b * 128
            blk = ew[h0:h0 + HL, t0:t0 + 128, sg:sg + 128]  # [hl, t, s]
            sub[:, :, j * 128:(j + 1) * 128] = blk.transpose(2, 0, 1)
        wt[:, off:off + HL * W] = sub.reshape(128, HL * W)
    wt = wt.astype(ml_dtypes.float8_e4m3)
    return {"xt": xt, "wkv": wkv, "wo": wo, "wt": wt}


def make_in_maps(x, Wk, bk, Wv, bv, w_aft, Wo, bo, has_bias):
    tt = np.arange(SEQ)
    causal = tt[:, None] >= tt[None, :]          # [t, s]
    ew = np.where(causal[None], np.exp(w_aft), 0.0).astype(np.float32)
    in_maps = []
    for core in range(8):
        n, g = core // 2, core % 2
        in_maps.append(pack_core(x[n], Wk, bk, Wv, bv, ew, Wo, bo, g,
                                 has_bias))
    return in_maps


def unscatter(results):
    out = np.empty((N_B, SEQ, DIM), np.float32)
    for core in range(8):
        n, g = core // 2, core % 2
        OWN = OB_A if g == 0 else OB_B
        r = np.asarray(results[core]["out"], np.float32)
        for j, b in enumerate(OWN):
            out[n, b * 128:(b + 1) * 128, :] = r[j * 128:(j + 1) * 128, :]
    return out


def _enable_tracing():
    """Best-effort: install the NTFF profile hook that this image's antenv
    lacks, so run_bass_kernel_spmd(trace=True) yields exec_time_ns."""
    import sys
    import types
    try:
        from antenv import axon_hooks  # noqa: F401
    except ImportError:
        m = types.ModuleType("antenv.axon_hooks")
        _h = [None]
        m.set_axon_ntff_profile_hook = lambda hook: _h.__setitem__(0, hook)
        m.get_axon_ntff_profile_hook = lambda: _h[0]
        sys.modules["antenv.axon_hooks"] = m
        import antenv
        antenv.axon_hooks = m
    from antenv import axon_hooks as ah
    if ah.get_axon_ntff_profile_hook() is None:
        from trn_agent_boot.trn_boot import _ntff_profile_via_ctypes
        ah.set_axon_ntff_profile_hook(
            _ntff_profile_via_ctypes("/opt/axon/libaxon_pjrt.so"))
    # artifact upload has no destination in this container; keep local only
    import concourse.bass_utils as bu
    bu.upload_artifacts = lambda tmpdir: tmpdir


def kernel(x, Wk, bk, Wv, bv, w_aft, Wo, bo):
    from concourse.bass_utils import run_bass_kernel_spmd

    global LAST_EXEC_NS, LAST_RESULTS
    x = np.asarray(x, np.float32)
    Wk = np.asarray(Wk, np.float32)
    bk = np.asarray(bk, np.float32)
    Wv = np.asarray(Wv, np.float32)
    bv = np.asarray(bv, np.float32)
    w_aft = np.asarray(w_aft, np.float32)
    Wo = np.asarray(Wo, np.float32)
    bo = np.asarray(bo, np.float32)
    has_bias = bool(np.any(bk) or np.any(bv) or np.any(bo))

    if os.environ.get("AFT_DEBUG_HOOK", "0") == "1":
        # surface python exceptions that the C++ compile callback swallows
        import traceback
        from concourse import bass2jax as _b2j
        _real = _b2j.neuronx_cc_hook

        def _loud(*a, **kw):
            try:
                return _real(*a, **kw)
            except BaseException:
                traceback.print_exc()
                raise

        _b2j.neuronx_cc_hook = _loud

    nc = build_nc(has_bias)
    in_maps = make_in_maps(x, Wk, bk, Wv, bv, w_aft, Wo, bo, has_bias)
    trace = os.environ.get("AFT_TRACE", "0") == "1"
    kw = {}
    if trace:
        try:
            _enable_tracing()
            kw["tmpdir"] = os.environ.get("AFT_TRACE_DIR") or None
        except Exception as e:  # profiling is best-effort only
            print(f"tracing unavailable: {e}")
            trace = False
    res = run_bass_kernel_spmd(nc, in_maps, core_ids=list(range(8)),
                               trace=trace, **kw)
    LAST_EXEC_NS = res.exec_time_ns
    LAST_RESULTS = res
    return unscatter(res.results)


# revision 19
# speedup vs baseline: 1.3519x; 1.3519x over previous
"""AFT (Attention-Free Transformer) distributed Bass kernel for 8 TRN2 NeuronCores.

Sharding: core = (batch n in 0..3) x (head-half g in 0..1). Each core projects
k/v for only its 4 heads (halving the projection matmul vs recompute), runs
the causal einsum for those heads over ALL 16 t-blocks, then a pairwise
AllGather (TOPSP/SDMA silicon, overlapped with einsum) exchanges aft halves
so each core can out-project its 8 owned t-blocks over all 8 heads.

SPMD symmetry: the einsum processes t-blocks in 16 slots = [send-set 8,
own-set 8]; per-parity block orders (OB_A/OB_B) are host-packed into the
w_aft slabs so both parities share one instruction stream with identical
padded causal extents (EQ). The out-proj contracts 12 tiles (4 local af +
8 AllGather tiles); host-zeroed Wo rows select the partner half per core.

exp(w_aft) (+ causal mask as exact zeros) is precomputed on host into the
slabs - no on-device exp for the einsum at all.

Per-core pipeline (matmuls bf16, fp32 PSUM):
  1. k/v proj (4 heads): k|v = xT.T @ [Wk|Wv][:, own cols] -> ek=exp(k),
     ekv=ek*v resident in SBUF.
  2. einsum per quad of 4 t-block slots: num/den[d,t] += (ekv|ek)[s,d].T @
     ew_slab[s, t-cols]; aft = num * (1/den) -> bf16.
  3. after send-set quads 0/1: DMA aft -> DRAM, chunked pairwise AllGather,
     DMA partner half -> SBUF (hidden under own-set einsum).
  4. out-proj: out[t,:] = sum_idx aft_tile_idx.T @ Wo_idx (12 tiles).

Self-contained: hardcodes all shapes for x[4,2048,1024], w_aft[8,2048,2048].
"""

import os

import numpy as np
import ml_dtypes

import concourse.bass as bass
import concourse.bacc as bacc
import concourse.mybir as mybir
import concourse.tile as tile

BF16 = ml_dtypes.bfloat16
P = 128
N_B, SEQ, DIM, H = 4, 2048, 1024, 8
HL = 4            # local heads per core
NT = 16           # 128-row t-blocks
HCOL = HL * P     # 512 k (or v) columns per core

# Per-parity t-block orders (from the baseline's balanced causal pairing):
# slots j and the EQ padded extents line up across parities so both cores
# share one instruction stream.
OB_A = [7, 4, 3, 0, 15, 12, 11, 8]
OB_B = [6, 5, 2, 1, 14, 13, 10, 9]
EQ8 = [8, 6, 4, 2, 16, 14, 12, 10]   # padded extent (s-tiles) per slot in a set
# 16 slots = send-set(8) + own-set(8); 4 quads of 4 slots.
EQ16 = EQ8 + EQ8
QUAD_ST = [8, 16, 8, 16]             # s-tile loop bound per quad


def _w_of(q, st):
    return P * sum(1 for j in range(4) if EQ16[4 * q + j] > st)


# Slab consumption order: (quad, s-tile PAIR) -> (W, col offset in wt).
# DoubleRow fp8 matmuls process two s-tiles per instruction; EQ entries are
# all even so W is constant within each pair.
SLABS = []
_off = 0
for _q in range(4):
    for _stp in range(QUAD_ST[_q] // 2):
        _W = _w_of(_q, 2 * _stp)
        assert _W == _w_of(_q, 2 * _stp + 1)
        SLABS.append((_q, _stp, _W, _off))
        _off += 2 * HL * _W
TOTC = _off  # 4 heads * 144 s-tile-blocks * 128 = 73728

LAST_EXEC_NS = None
LAST_RESULTS = None


def build_nc(has_bias):
    NIT = 9 if has_bias else 8   # k-tiles in the x^T contraction (+1 bias row)
    NHO = 13 if has_bias else 12  # out-proj contraction tiles (+1 bias row)
    SXT = NIT * 128
    F32 = mybir.dt.float32
    BF = mybir.dt.bfloat16
    F8 = mybir.dt.float8e4
    EXP = mybir.ActivationFunctionType.Exp
    RG = [[0, 1], [2, 3], [4, 5], [6, 7]]

    nc = bacc.Bacc("TRN2", target_bir_lowering=False, num_devices=8)
    xt_d = nc.declare_dram_parameter("xt", [16, P, SXT], BF, isOutput=False)
    wkv_d = nc.declare_dram_parameter("wkv", [NIT, P, 1024], BF, isOutput=False)
    wo_d = nc.declare_dram_parameter("wo", [P, NHO, 1024], BF, isOutput=False)
    wt_d = nc.declare_dram_parameter("wt", [P, TOTC], F8, isOutput=False)
    out_d = nc.declare_dram_parameter("out", [1024, 1024], F32, isOutput=True)

    with tile.TileContext(nc) as tc:
        with tc.tile_pool(name="res", bufs=1) as res, \
             tc.tile_pool(name="aftp", bufs=16) as aftp, \
             tc.tile_pool(name="wop", bufs=1) as wop, \
             tc.tile_pool(name="wkvp", bufs=1) as wkvp, \
             tc.tile_pool(name="wtr", bufs=10) as wtr, \
             tc.tile_pool(name="recp", bufs=2) as recp, \
             tc.tile_pool(name="outp", bufs=3) as outp, \
             tc.tile_pool(name="agd", bufs=4, space="DRAM") as agd:
            ek_sb = res.tile([P, 16, HCOL], F8, name="ek_sb")
            ekv_sb = res.tile([P, 16, HCOL], F8, name="ekv_sb")
            xt_sb = res.tile([P, 16, SXT], BF, name="xt_sb")
            ag_sb = res.tile([P, 8, 1024], BF, name="ag_sb")
            wo_sb = wop.tile([P, NHO, 1024], BF, name="wo_sb")
            wkv_sb = wkvp.tile([P, NIT, 1024], BF, name="wkv_sb")
            agin_a = agd.tile([512, 512], BF, name="agin_a")
            agin_b = agd.tile([512, 512], BF, name="agin_b")
            agout_a = agd.tile([1024, 512], BF, name="agout_a")
            agout_b = agd.tile([1024, 512], BF, name="agout_b")

            # first-needed inputs lead their queues: wkv[0] on sync,
            # xt[0] on gpsimd; wo (needed only in phase 3) is issued after
            # the phase-1 loop so it doesn't contend with startup DMAs.
            nc.gpsimd.dma_start(out=xt_sb[:, 0, :], in_=xt_d[0, :, :])
            for i in range(NIT):
                nc.sync.dma_start(out=wkv_sb[:, i, :], in_=wkv_d[i, :, :])
            # -ln(2) bias for the Exp activation: ek is stored as exp(k)/2
            # so fp8e4m3's max-normal (240) is never exceeded.
            nln2 = res.tile([P, 1], F32, name="nln2")
            nc.vector.memset(nln2[:, :], -0.6931471805599453)
            ones_t = None
            if has_bias:
                ones_t = res.tile([P, P], BF, name="ones_t")
                nc.vector.memset(ones_t[:, :], 0.0)
                nc.vector.memset(ones_t[0:1, :], 1.0)

            # ---------------- phase 1: k/v projection (own 4 heads) --------
            with tc.tile_pool(name="pkv", bufs=6, space="PSUM") as pkv:
                for st in range(16):
                    if st > 0:
                        nc.sync.dma_start(out=xt_sb[:, st, :],
                                          in_=xt_d[st, :, :])
                    kp = pkv.tile([P, HCOL], F32, name="kp", tag="pkv")
                    vp = pkv.tile([P, HCOL], F32, name="vp", tag="pkv")
                    for it in range(NIT):
                        lh = xt_sb[:, st, it * 128:(it + 1) * 128]
                        s0 = it == 0
                        s1 = it == NIT - 1
                        nc.tensor.matmul(kp[:, :], lh,
                                         wkv_sb[:, it, 0:HCOL],
                                         start=s0, stop=s1)
                        nc.tensor.matmul(vp[:, :], lh,
                                         wkv_sb[:, it, HCOL:1024],
                                         start=s0, stop=s1)
                    nc.scalar.activation(ek_sb[:, st, :], kp[:, :], EXP,
                                         bias=nln2[:, :])
                    nc.vector.tensor_mul(ekv_sb[:, st, :], vp[:, :],
                                         ek_sb[:, st, :])
                for i in range(NHO):
                    nc.scalar.dma_start(out=wo_sb[:, i, :], in_=wo_d[:, i, :])

            # ------------- phase 2: einsum + aft exchange ------------------
            aft = {}
            si = 0
            with tc.tile_pool(name="pe", bufs=8, space="PSUM") as pep:
                for q in range(4):
                    nd = []
                    for hh in range(HL):
                        nt = pep.tile([P, 512], F32, name="ps_n", tag="ps")
                        dn = pep.tile([P, 512], F32, name="ps_d", tag="ps")
                        nd.append((nt, dn))
                    NPAIR = QUAD_ST[q] // 2
                    for stp in range(NPAIR):
                        q_, stp_, W, off = SLABS[si]
                        si += 1
                        assert (q_, stp_) == (q, stp)
                        slab = wtr.tile([P, 2, 2048], F8, name="slab",
                                        tag="slab")
                        nc.sync.dma_start(
                            out=slab[:, 0, 0:HL * W],
                            in_=wt_d[:, off:off + HL * W])
                        nc.sync.dma_start(
                            out=slab[:, 1, 0:HL * W],
                            in_=wt_d[:, off + HL * W:off + 2 * HL * W])
                        s0 = stp == 0
                        s1 = stp == NPAIR - 1
                        sp = slice(2 * stp, 2 * stp + 2)
                        for hh in range(HL):
                            nt, dn = nd[hh]
                            rhs = slab[:, :, hh * W:(hh + 1) * W]
                            nc.tensor.matmul(
                                nt[:, 0:W],
                                ekv_sb[:, sp, hh * 128:(hh + 1) * 128],
                                rhs, start=s0, stop=s1,
                                perf_mode=mybir.MatmulPerfMode.DoubleRow)
                            nc.tensor.matmul(
                                dn[:, 0:W],
                                ek_sb[:, sp, hh * 128:(hh + 1) * 128],
                                rhs, start=s0, stop=s1,
                                perf_mode=mybir.MatmulPerfMode.DoubleRow)
                    for hh in range(HL):
                        nt, dn = nd[hh]
                        rc = recp.tile([P, 512], F32, name="rc", tag="rc")
                        nc.vector.reciprocal_approx_fast(rc[:, :], dn[:, :])
                        af = aftp.tile([P, 512], BF, name="af", tag="af")
                        nc.vector.tensor_mul(af[:, :], nt[:, :], rc[:, :])
                        aft[(q, hh)] = af

            # Collectives are emitted AFTER the whole einsum: the tile
            # framework serializes later-emitted DMAs behind in-flight
            # collectives, so any slab DMA after an AG would stall the PE.
            # Data deps (af tiles -> agin -> AG -> agout -> ag_sb) still
            # order these correctly at runtime.
            for q in range(2):
                agin = agin_a if q == 0 else agin_b
                for hh in range(HL):
                    nc.scalar.dma_start(
                        out=agin[hh * 128:(hh + 1) * 128, :],
                        in_=aft[(q, hh)][:, :])
            for q in range(2):
                agin = agin_a if q == 0 else agin_b
                agout = agout_a if q == 0 else agout_b
                nc.gpsimd.collective_compute(
                    "AllGather", mybir.AluOpType.bypass,
                    replica_groups=RG,
                    ins=[agin.opt()], outs=[agout.opt()])
            for q in range(2):
                agout = agout_a if q == 0 else agout_b
                c0 = q * 512
                for half in range(2):
                    for hh in range(HL):
                        r0 = half * 512 + hh * 128
                        nc.scalar.dma_start(
                            out=ag_sb[:, half * 4 + hh, c0:c0 + 512],
                            in_=agout[r0:r0 + 128, :])

            # ---------------- phase 3: out-projection ----------------------
            with tc.tile_pool(name="po", bufs=2, space="PSUM") as pop:
                for tb in range(8):
                    q = 2 + tb // 4
                    jj = tb % 4
                    ops = pop.tile([P, 1024], F32, name="ps_o", tag="po")
                    for idx in range(NHO):
                        if idx < 4:
                            lh = aft[(q, idx)][:, jj * 128:(jj + 1) * 128]
                        elif idx < 12:
                            lh = ag_sb[:, idx - 4, tb * 128:(tb + 1) * 128]
                        else:
                            lh = ones_t[:, :]
                        s0 = idx == 0
                        s1 = idx == NHO - 1
                        nc.tensor.matmul(
                            ops[:, 0:512], lh, wo_sb[:, idx, 0:512],
                            start=s0, stop=s1)
                        nc.tensor.matmul(
                            ops[:, 512:1024], lh, wo_sb[:, idx, 512:1024],
                            start=s0, stop=s1)
                    osb = outp.tile([P, 1024], F32, name="osb", tag="osb")
                    nc.vector.tensor_copy(osb[:, :], ops[:, :])
                    nc.sync.dma_start(
                        out=out_d[tb * 128:(tb + 1) * 128, :], in_=osb[:, :])
    nc.compile()
    return nc


def pack_core(xn, Wk, bk, Wv, bv, ew, Wo, bo, g, has_bias):
    """Per-core input map (layout transforms + bf16 casts). ew = masked
    exp(w_aft) [8, T, S] fp32, computed once by the caller."""
    SEND = OB_B if g == 0 else OB_A
    OWN = OB_A if g == 0 else OB_B
    slots = SEND + OWN
    h0 = HL * g

    # x^T tiles: xt[st, p, it*128+ss] = x[st*128+ss, it*128+p]
    xr = xn.reshape(16, 128, 8, 128).transpose(0, 3, 2, 1)  # [st, p, it, ss]
    xt = np.ascontiguousarray(xr).reshape(16, 128, 1024)
    if has_bias:
        aug = np.zeros((16, 128, 128), np.float32)
        aug[:, 0, :] = 1.0
        xt = np.concatenate([xt, aug], axis=2)
    xt = xt.astype(BF16)

    wkv = np.concatenate([Wk[:, h0 * 128:(h0 + HL) * 128],
                          Wv[:, h0 * 128:(h0 + HL) * 128] * 0.25],
                         axis=1).reshape(8, 128, 1024)
    if has_bias:
        aug = np.zeros((1, 128, 1024), np.float32)
        aug[0, 0, :] = np.concatenate([bk[h0 * 128:(h0 + HL) * 128],
                                       bv[h0 * 128:(h0 + HL) * 128] * 0.25])
        wkv = np.concatenate([wkv, aug], axis=0)
    wkv = wkv.astype(BF16)

    # out-proj tiles: 0-3 own heads, 4-7 AG lo half (global h0-3),
    # 8-11 AG hi half (global h4-7); partner rows real, self-echo zero.
    wor = (Wo * 4.0).reshape(8, 128, 1024)
    wo = np.zeros((12, 128, 1024), np.float32)
    wo[0:4] = wor[h0:h0 + HL]
    if g == 0:
        wo[8:12] = wor[4:8]
    else:
        wo[4:8] = wor[0:4]
    if has_bias:
        aug = np.zeros((1, 128, 1024), np.float32)
        aug[0, 0, :] = bo
        wo = np.concatenate([wo, aug], axis=0)
    wo = np.ascontiguousarray(wo.transpose(1, 0, 2)).astype(BF16)

    # host-precomputed exp(w_aft) slabs, causally packed & transposed,
    # s-tile PAIRS back to back for DoubleRow
    wt = np.empty((128, TOTC), np.float32)
    for (q, stp, W, off) in SLABS:
        cnt = W // 128
        for j2 in range(2):
            st = 2 * stp + j2
            sub = np.zeros((128, HL, W), np.float32)
            sg = st * 128
            for j in range(cnt):
                b = slots[4 * q + j]
                t0 = b * 128
                blk = ew[h0:h0 + HL, t0:t0 + 128, sg:sg + 128]  # [hl, t, s]
                sub[:, :, j * 128:(j + 1) * 128] = blk.transpose(2, 0, 1)
            o = off + j2 * HL * W
            wt[:, o:o + HL * W] = sub.reshape(128, HL * W)
    wt = wt.astype(ml_dtypes.float8_e4m3)
    return {"xt": xt, "wkv": wkv, "wo": wo, "wt": wt}


def make_in_maps(x, Wk, bk, Wv, bv, w_aft, Wo, bo, has_bias):
    tt = np.arange(SEQ)
    causal = tt[:, None] >= tt[None, :]          # [t, s]
    ew = np.where(causal[None], np.exp(w_aft), 0.0).astype(np.float32)
    in_maps = []
    for core in range(8):
        n, g = core // 2, core % 2
        in_maps.append(pack_core(x[n], Wk, bk, Wv, bv, ew, Wo, bo, g,
                                 has_bias))
    return in_maps


def unscatter(results):
    out = np.empty((N_B, SEQ, DIM), np.float32)
    for core in range(8):
        n, g = core // 2, core % 2
        OWN = OB_A if g == 0 else OB_B
        r = np.asarray(results[core]["out"], np.float32)
        for j, b in enumerate(OWN):
            out[n, b * 128:(b + 1) * 128, :] = r[j * 128:(j + 1) * 128, :]
    return out


def _enable_tracing():
    """Best-effort: install the NTFF profile hook that this image's antenv
    lacks, so run_bass_kernel_spmd(trace=True) yields exec_time_ns."""
    import sys
    import types
    try:
        from antenv import axon_hooks  # noqa: F401
    except ImportError:
        m = types.ModuleType("antenv.axon_hooks")
        _h = [None]
        m.set_axon_ntff_profile_hook = lambda hook: _h.__setitem__(0, hook)
        m.get_axon_ntff_profile_hook = lambda: _h[0]
        sys.modules["antenv.axon_hooks"] = m
        import antenv
        antenv.axon_hooks = m
    from antenv import axon_hooks as ah
    if ah.get_axon_ntff_profile_hook() is None:
        from trn_agent_boot.trn_boot import _ntff_profile_via_ctypes
        ah.set_axon_ntff_profile_hook(
            _ntff_profile_via_ctypes("/opt/axon/libaxon_pjrt.so"))
    # artifact upload has no destination in this container; keep local only
    import concourse.bass_utils as bu
    bu.upload_artifacts = lambda tmpdir: tmpdir


def kernel(x, Wk, bk, Wv, bv, w_aft, Wo, bo):
    from concourse.bass_utils import run_bass_kernel_spmd

    global LAST_EXEC_NS, LAST_RESULTS
    x = np.asarray(x, np.float32)
    Wk = np.asarray(Wk, np.float32)
    bk = np.asarray(bk, np.float32)
    Wv = np.asarray(Wv, np.float32)
    bv = np.asarray(bv, np.float32)
    w_aft = np.asarray(w_aft, np.float32)
    Wo = np.asarray(Wo, np.float32)
    bo = np.asarray(bo, np.float32)
    has_bias = bool(np.any(bk) or np.any(bv) or np.any(bo))

    if os.environ.get("AFT_DEBUG_HOOK", "0") == "1":
        # surface python exceptions that the C++ compile callback swallows
        import traceback
        from concourse import bass2jax as _b2j
        _real = _b2j.neuronx_cc_hook

        def _loud(*a, **kw):
            try:
                return _real(*a, **kw)
            except BaseException:
                traceback.print_exc()
                raise

        _b2j.neuronx_cc_hook = _loud

    nc = build_nc(has_bias)
    in_maps = make_in_maps(x, Wk, bk, Wv, bv, w_aft, Wo, bo, has_bias)
    trace = os.environ.get("AFT_TRACE", "0") == "1"
    kw = {}
    if trace:
        try:
            _enable_tracing()
            kw["tmpdir"] = os.environ.get("AFT_TRACE_DIR") or None
        except Exception as e:  # profiling is best-effort only
            print(f"tracing unavailable: {e}")
            trace = False
    res = run_bass_kernel_spmd(nc, in_maps, core_ids=list(range(8)),
                               trace=trace, **kw)
    LAST_EXEC_NS = res.exec_time_ns
    LAST_RESULTS = res
    return unscatter(res.results)
